# revision 12
# baseline (speedup 1.0000x reference)
"""Trainium2 Bass kernel for the merged multi-adapter LoRA layer.

Math (all fp32 reference):
    t[n,b,j,d]  = sum_m x[b,j,m] * lora_A[n,d,m]
    out[n,b,j,k] = sum_d t[n,b,j,d] * lora_B[n,k,d]

Shapes: x (4,2048,4096), lora_A (4,16,4096), lora_B (4,4096,16)
        out (4,4,2048,4096)

Sharding: data-parallel over flattened tokens (b*j = 8192 -> 1024/core on
8 cores); the tiny LoRA params are replicated.

Per-core HBM traffic: 8 MiB x (f16 in) + 32 MiB out (f16, widened on host)
+ ~2 MiB params  ->  ~117 us at 358 GB/s.

The PE HAM clock-gate throttles the PE to 1.2 GHz for most of the run
(a ~25 us warm allowance early, then an imposed 50% util limit), so the
design keeps the PE OFF the critical path entirely via array packing:

  - mm2 runs in 32x128 row-tiled mode: the four adapters' D=16
    contractions execute CONCURRENTLY on four 32-row PE tiles (adapter n
    reads its t/B slices from SBUF partitions 32n..32n+15, writes its own
    PSUM bank).  A group of four 512-wide matmuls costs ~one matmul's
    cycles, so mm2 drops from 131k to ~35k PE cycles.
  - mm1 (chunk c) stays in full 128x128 mode as a back-to-back 32-matmul
    accumulation chain between chunks (mode switches force a PE drain, so
    the chain is contiguous, not woven into mm2).
  - adapters are paired into [128,1024] PSUM tiles (n0|n1, n2|n3) so the
    f32->f16 evacuation runs as 1024-wide copies alternating
    Vector/Scalar; output staging interleaves kg-blocks of the two
    adapters and the stores un-stride them (1 KiB rows).
  - x arrives pre-transposed/packed as [chunk, half, 128, 8, 512] f16,
    one 1 MiB DMA per half-chunk split across the Scalar/Sync trigger
    queues; scratch-tile warm-up matmuls bridge the initial load.
  - stores issue per (adapter, half-row-block): 0.5 MiB each, starting
    ~4 matmul groups into the first strip.
"""

import numpy as np

import concourse.bacc as bacc
import concourse.bass as bass
import concourse.mybir as mybir
import concourse.tile as tile
from concourse import bass_utils
from concourse.bass import ds, ts

F32 = mybir.dt.float32
F16 = mybir.dt.float16

N_CORES = 8
B, J, M = 4, 2048, 4096
N, D, K = 4, 16, 4096
TOK = B * J                  # 8192 flattened tokens
TPC = TOK // N_CORES         # 1024 tokens per core
CH = 256                     # token chunk (mm1 granularity)
NCH = TPC // CH              # 4
N_MT = M // 128              # 32 m-tiles
NPAIR = N_MT // 2            # 16 packed m-tile pairs
NPH = NPAIR // 2             # pairs per half-chunk DMA (8)
KT = 512                     # mm2 matmul free width
NKG = K // KT                # 8 column groups
ADP = 32                     # partition stride per adapter in the packed dim
NSTRIP = CH // 128           # 128-token strips per chunk (2)
WARMUP = 16                  # scratch matmuls to un-throttle the PE HAM


def build_program():
    nc = bacc.Bacc("TRN2")

    xs = nc.dram_tensor(
        "xs", [NCH, 2, 128, NPH, 2 * CH], F16, kind="ExternalInput"
    ).ap()
    a_p = nc.dram_tensor("a_p", [128, N_MT, 128], F16, kind="ExternalInput").ap()
    b_p = nc.dram_tensor("b_p", [128, K], F16, kind="ExternalInput").ap()
    o = nc.dram_tensor("o", [N, TPC, K], F16, kind="ExternalOutput").ap()

    with tile.TileContext(nc) as tc:
        with (
            tc.tile_pool(name="apool", bufs=1) as apool,
            tc.tile_pool(name="bpool", bufs=1) as bpool,
            tc.tile_pool(name="spool", bufs=1) as spool,
            tc.tile_pool(name="xpool", bufs=2 * NCH) as xpool,
            tc.tile_pool(name="tpool", bufs=2) as tpool,
            tc.tile_pool(name="opool", bufs=6) as opool,
            tc.tile_pool(name="tps", bufs=1, space="PSUM") as tps_pool,
            tc.tile_pool(name="ops", bufs=3, space="PSUM") as ops_pool,
        ):
            xsb = {}
            for c in range(NCH):
                for h in range(2):
                    xsb[(c, h)] = xpool.tile([128, NPH, 2 * CH], F16, tag="x", name="x")
            a_sb = apool.tile([128, N_MT, 128], F16, tag="a")
            b_sb = bpool.tile([128, K], F16, tag="b")

            # the two halves of each chunk load in parallel on the two
            # hardware-DGE trigger queues (Scalar and Sync)
            nc.scalar.dma_start(xsb[(0, 0)][:], xs[0, 0])
            nc.sync.dma_start(a_sb[:], a_p[:])
            nc.sync.dma_start(xsb[(0, 1)][:], xs[0, 1])
            nc.scalar.dma_start(xsb[(1, 0)][:], xs[1, 0])
            nc.sync.dma_start(b_sb[:], b_p[:])
            nc.sync.dma_start(xsb[(1, 1)][:], xs[1, 1])
            nc.scalar.dma_start(xsb[(2, 0)][:], xs[2, 0])
            nc.sync.dma_start(xsb[(2, 1)][:], xs[2, 1])
            nc.scalar.dma_start(xsb[(3, 0)][:], xs[3, 0])
            nc.sync.dma_start(xsb[(3, 1)][:], xs[3, 1])

            scr = spool.tile([128, KT], F16, tag="s", name="scr")
            nc.vector.memset(scr[:], 0.0)

            # HAM warm-up on the dependency-free scratch tile while the
            # first x chunk streams in (PSUM dest reuses the tps bank)
            for _ in range(WARMUP):
                w_ps = tps_pool.tile([128, CH], F32, tag="tps", name="wps")
                nc.tensor.matmul(w_ps[:], lhsT=scr[:, ds(0, 128)],
                                 rhs=scr[:, ds(0, CH)],
                                 start=True, stop=True, skip_group_check=True)

            def mm1_chain(c):
                t_ps = tps_pool.tile([128, CH], F32, tag="tps", name="tps")
                for mt in range(N_MT):
                    nc.tensor.matmul(
                        t_ps[:],
                        lhsT=a_sb[:, mt, :],
                        rhs=xsb[(c, mt // 16)][:, (mt // 2) % NPH, ds((mt % 2) * CH, CH)],
                        start=(mt == 0),
                        stop=(mt == N_MT - 1),
                        skip_group_check=True,
                    )
                t_sb = tpool.tile([128, CH], F16, tag="t", name="t")
                nc.vector.tensor_copy(t_sb[:], t_ps[:])
                return t_sb

            t_sb_next = mm1_chain(0)

            for c in range(NCH):
                t_sb = t_sb_next

                for s in range(NSTRIP):
                    # two staging tiles per strip; kg-blocks interleave the
                    # adapter pair: [kg, (n_even 512 | n_odd 512)]
                    osb = [
                        opool.tile([128, NKG, 2 * KT], F16, tag="o", name="osb")
                        for _ in range(2)
                    ]
                    for kg in range(NKG):
                        o_ps = [
                            ops_pool.tile([128, 2 * KT], F32, tag="ops", name="ops")
                            for _ in range(2)
                        ]
                        # 4 adjacent matmuls on 4 distinct 32-row PE tiles
                        # -> they stream concurrently (~1 matmul of cycles)
                        for n in range(N):
                            nc.tensor.matmul(
                                o_ps[n // 2][:, ts(n % 2, KT)],
                                lhsT=t_sb[ds(ADP * n, D), ts(s, 128)],
                                rhs=b_sb[ds(ADP * n, D), ts(kg, KT)],
                                start=True,
                                stop=True,
                                tile_position=(ADP * n, 0),
                                skip_group_check=True,
                            )
                        if kg % 2 == 0:
                            nc.vector.tensor_copy(osb[0][:, kg], o_ps[0][:])
                            nc.scalar.copy(osb[1][:, kg], o_ps[1][:])
                        else:
                            nc.scalar.copy(osb[0][:, kg], o_ps[0][:])
                            nc.vector.tensor_copy(osb[1][:, kg], o_ps[1][:])

                        # half-osb stores: 4 kg-blocks per store, strided
                        # out of the interleaved staging layout
                        if kg % 4 == 3:
                            h = kg // 4
                            for n in range(N):
                                nc.sync.dma_start(
                                    o[n, ds(c * CH + s * 128, 128),
                                      ds(h * 4 * KT, 4 * KT)],
                                    osb[n // 2][:, ds(4 * h, 4), ts(n % 2, KT)],
                                )

                    # next chunk's mm1 chain mid-chunk (its x has landed by
                    # now); contiguous so the 128x128<->32x128 PE mode
                    # switch only drains twice per chunk
                    if s == 1 and c + 1 < NCH:
                        t_sb_next = mm1_chain(c + 1)

    nc.compile()
    return nc


_NC_CACHE = []


def _get_nc():
    if not _NC_CACHE:
        _NC_CACHE.append(build_program())
    return _NC_CACHE[0]


def prepare_inputs(x, lora_A, lora_B):
    x = np.ascontiguousarray(np.asarray(x, dtype=np.float32)).astype(np.float16)
    lora_A = np.asarray(lora_A, dtype=np.float32)
    lora_B = np.asarray(lora_B, dtype=np.float32)

    xf = x.reshape(TOK, M)

    # a_t[m, 32n+d] = lora_A[n, d, m]; packed to [p, mt, c] so each SBUF
    # partition reads one contiguous row.
    a_t = np.zeros((M, 128), dtype=np.float32)
    for n in range(N):
        a_t[:, ADP * n : ADP * n + D] = lora_A[n].T
    a_pack = np.ascontiguousarray(
        a_t.reshape(N_MT, 128, 128).transpose(1, 0, 2)
    ).astype(np.float16)

    # b_pad[32n+d, k] = lora_B[n, k, d]
    b_pad = np.zeros((128, K), dtype=np.float16)
    for n in range(N):
        b_pad[ADP * n : ADP * n + D, :] = lora_B[n].T

    in_maps = []
    for c in range(N_CORES):
        # xp[chunk, half, p, pq, sub*CH + t] = x^T[(2*(8h+pq)+sub)*128 + p,
        #                                          chunk*CH + t]
        xT = xf[c * TPC : (c + 1) * TPC].T                  # [M, TPC]
        xr = xT.reshape(2, NPH, 2, 128, NCH, CH)            # [h, pq, sub, p, ch, t]
        xp = np.ascontiguousarray(xr.transpose(4, 0, 3, 1, 2, 5)).reshape(
            NCH, 2, 128, NPH, 2 * CH
        )
        in_maps.append({"xs": xp, "a_p": a_pack, "b_p": b_pad})
    return in_maps


def run(x, lora_A, lora_B, trace=False, **spmd_kwargs):
    nc = _get_nc()
    in_maps = prepare_inputs(x, lora_A, lora_B)
    res = bass_utils.run_bass_kernel_spmd(
        nc, in_maps, list(range(N_CORES)), trace=trace, **spmd_kwargs
    )
    o_full = np.concatenate(
        [res.results[c]["o"].astype(np.float32) for c in range(N_CORES)], axis=1
    )
    return o_full.reshape(N, B, J, K), res


def kernel(x, lora_A, lora_B):
    out, _ = run(x, lora_A, lora_B)
    return out


# revision 15
# speedup vs baseline: 1.1397x; 1.1397x over previous
"""Trainium2 Bass kernel for the merged multi-adapter LoRA layer.

Math (all fp32 reference):
    t[n,b,j,d]  = sum_m x[b,j,m] * lora_A[n,d,m]
    out[n,b,j,k] = sum_d t[n,b,j,d] * lora_B[n,k,d]

Shapes: x (4,2048,4096), lora_A (4,16,4096), lora_B (4,4096,16)
        out (4,4,2048,4096)

Sharding: data-parallel over flattened tokens (b*j = 8192 -> 1024/core on
8 cores); the tiny LoRA params are replicated.

Per-core HBM traffic: 8 MiB x (f16 in) + 32 MiB out (f16, widened on host)
+ ~2 MiB params  ->  ~117 us at 358 GB/s.

The PE HAM clock-gate throttles the PE to 1.2 GHz for most of the run
(a ~25 us warm allowance early, then an imposed 50% util limit), so the
design keeps the PE OFF the critical path entirely via array packing:

  - mm2 runs in 32x128 row-tiled mode: the four adapters' D=16
    contractions execute CONCURRENTLY on four 32-row PE tiles (adapter n
    reads its t/B slices from SBUF partitions 32n..32n+15, writes its own
    PSUM bank).  A group of four 512-wide matmuls costs ~one matmul's
    cycles, so mm2 drops from 131k to ~35k PE cycles.
  - mm1 (chunk c) stays in full 128x128 mode as a back-to-back 32-matmul
    accumulation chain between chunks (mode switches force a PE drain, so
    the chain is contiguous, not woven into mm2).
  - adapters are paired into [128,1024] PSUM tiles (n0|n1, n2|n3) so the
    f32->f16 evacuation runs as 1024-wide copies alternating
    Vector/Scalar; output staging interleaves kg-blocks of the two
    adapters and the stores un-stride them (1 KiB rows).
  - x arrives pre-transposed/packed as [chunk, half, 128, 8, 512] f16,
    one 1 MiB DMA per half-chunk split across the Scalar/Sync trigger
    queues; scratch-tile warm-up matmuls bridge the initial load.
  - stores issue per (adapter, half-row-block): 0.5 MiB each, starting
    ~4 matmul groups into the first strip.
"""

import numpy as np

import concourse.bacc as bacc
import concourse.bass as bass
import concourse.mybir as mybir
import concourse.tile as tile
from concourse import bass_utils
from concourse.bass import ds, ts

F32 = mybir.dt.float32
F16 = mybir.dt.float16

N_CORES = 8
B, J, M = 4, 2048, 4096
N, D, K = 4, 16, 4096
TOK = B * J                  # 8192 flattened tokens
TPC = TOK // N_CORES         # 1024 tokens per core
CH = 256                     # token chunk (mm1 granularity)
NCH = TPC // CH              # 4
N_MT = M // 128              # 32 m-tiles
NPAIR = N_MT // 2            # 16 packed m-tile pairs
NPH = NPAIR // 2             # pairs per half-chunk DMA (8)
KT = 512                     # mm2 matmul free width
NKG = K // KT                # 8 column groups
ADP = 32                     # partition stride per adapter in the packed dim
NSTRIP = CH // 128           # 128-token strips per chunk (2)
WARMUP = 8                   # scratch matmuls to un-throttle the PE HAM


def build_program():
    nc = bacc.Bacc("TRN2")

    xs = nc.dram_tensor(
        "xs", [NCH, 2, 128, NPH, 2 * CH], F16, kind="ExternalInput"
    ).ap()
    a_p = nc.dram_tensor("a_p", [128, N_MT, 128], F16, kind="ExternalInput").ap()
    b_p = nc.dram_tensor("b_p", [128, K], F16, kind="ExternalInput").ap()
    o = nc.dram_tensor("o", [N, TPC, K], F16, kind="ExternalOutput").ap()

    with tile.TileContext(nc) as tc:
        with (
            tc.tile_pool(name="apool", bufs=1) as apool,
            tc.tile_pool(name="bpool", bufs=1) as bpool,
            tc.tile_pool(name="spool", bufs=1) as spool,
            tc.tile_pool(name="xpool", bufs=2 * NCH) as xpool,
            tc.tile_pool(name="tpool", bufs=2) as tpool,
            tc.tile_pool(name="opool", bufs=12) as opool,
            tc.tile_pool(name="tps", bufs=1, space="PSUM") as tps_pool,
            tc.tile_pool(name="ops", bufs=7, space="PSUM") as ops_pool,
        ):
            xsb = {}
            for c in range(NCH):
                for h in range(2):
                    xsb[(c, h)] = xpool.tile([128, NPH, 2 * CH], F16, tag="x", name="x")
            a_sb = apool.tile([128, N_MT, 128], F16, tag="a")
            b_sb = bpool.tile([128, K], F16, tag="b")

            # the two halves of each chunk load in parallel on the two
            # hardware-DGE trigger queues (Scalar and Sync)
            nc.scalar.dma_start(xsb[(0, 0)][:], xs[0, 0])
            nc.sync.dma_start(a_sb[:], a_p[:])
            nc.sync.dma_start(xsb[(0, 1)][:], xs[0, 1])
            nc.scalar.dma_start(xsb[(1, 0)][:], xs[1, 0])
            nc.sync.dma_start(b_sb[:], b_p[:])
            nc.sync.dma_start(xsb[(1, 1)][:], xs[1, 1])
            nc.scalar.dma_start(xsb[(2, 0)][:], xs[2, 0])
            nc.sync.dma_start(xsb[(2, 1)][:], xs[2, 1])
            nc.scalar.dma_start(xsb[(3, 0)][:], xs[3, 0])
            nc.sync.dma_start(xsb[(3, 1)][:], xs[3, 1])

            scr = spool.tile([128, KT], F16, tag="s", name="scr")
            nc.vector.memset(scr[:], 0.0)

            # HAM warm-up on the dependency-free scratch tile while the
            # first x chunk streams in (PSUM dest reuses the tps bank)
            for _ in range(WARMUP):
                w_ps = tps_pool.tile([128, CH], F32, tag="tps", name="wps")
                nc.tensor.matmul(w_ps[:], lhsT=scr[:, ds(0, 128)],
                                 rhs=scr[:, ds(0, CH)],
                                 start=True, stop=True, skip_group_check=True)

            def mm1_chain(c):
                t_ps = tps_pool.tile([128, CH], F32, tag="tps", name="tps")
                for mt in range(N_MT):
                    nc.tensor.matmul(
                        t_ps[:],
                        lhsT=a_sb[:, mt, :],
                        rhs=xsb[(c, mt // 16)][:, (mt // 2) % NPH, ds((mt % 2) * CH, CH)],
                        start=(mt == 0),
                        stop=(mt == N_MT - 1),
                        skip_group_check=True,
                    )
                t_sb = tpool.tile([128, CH], F16, tag="t", name="t")
                nc.vector.tensor_copy(t_sb[:], t_ps[:])
                return t_sb

            t_sb_next = mm1_chain(0)

            for c in range(NCH):
                t_sb = t_sb_next

                for s in range(NSTRIP):
                    osb = [
                        opool.tile([128, K], F16, tag="o", name="osb")
                        for _ in range(N)
                    ]
                    for kg in range(NKG):
                        o_ps = [
                            ops_pool.tile([128, KT], F32, tag="ops", name="ops")
                            for _ in range(N)
                        ]
                        # 4 adjacent matmuls on 4 distinct 32-row PE tiles
                        # (4 distinct PSUM banks) -> they stream
                        # concurrently, ~one matmul's cycles for all four
                        for n in range(N):
                            nc.tensor.matmul(
                                o_ps[n][:],
                                lhsT=t_sb[ds(ADP * n, D), ts(s, 128)],
                                rhs=b_sb[ds(ADP * n, D), ts(kg, KT)],
                                start=True,
                                stop=True,
                                tile_position=(ADP * n, 0),
                                skip_group_check=True,
                            )
                        for n in range(N):
                            if (kg + n) % 2 == 0:
                                nc.vector.tensor_copy(osb[n][:, ts(kg, KT)], o_ps[n][:])
                            else:
                                nc.scalar.copy(osb[n][:, ts(kg, KT)], o_ps[n][:])

                        # contiguous 0.5 MiB half-stores keep the wire busy
                        # from four matmul groups into the strip
                        if kg % 4 == 3:
                            h = kg // 4
                            for n in range(N):
                                nc.sync.dma_start(
                                    o[n, ds(c * CH + s * 128, 128),
                                      ds(h * 4 * KT, 4 * KT)],
                                    osb[n][:, ds(h * 4 * KT, 4 * KT)],
                                )

                    # next chunk's mm1 chain mid-chunk (its x has landed by
                    # now); contiguous so the 128x128<->32x128 PE mode
                    # switch only drains twice per chunk
                    if s == 0 and c + 1 < NCH:
                        t_sb_next = mm1_chain(c + 1)

    nc.compile()
    return nc


_NC_CACHE = []


def _get_nc():
    if not _NC_CACHE:
        _NC_CACHE.append(build_program())
    return _NC_CACHE[0]


def prepare_inputs(x, lora_A, lora_B):
    x = np.ascontiguousarray(np.asarray(x, dtype=np.float32)).astype(np.float16)
    lora_A = np.asarray(lora_A, dtype=np.float32)
    lora_B = np.asarray(lora_B, dtype=np.float32)

    xf = x.reshape(TOK, M)

    # a_t[m, 32n+d] = lora_A[n, d, m]; packed to [p, mt, c] so each SBUF
    # partition reads one contiguous row.
    a_t = np.zeros((M, 128), dtype=np.float32)
    for n in range(N):
        a_t[:, ADP * n : ADP * n + D] = lora_A[n].T
    a_pack = np.ascontiguousarray(
        a_t.reshape(N_MT, 128, 128).transpose(1, 0, 2)
    ).astype(np.float16)

    # b_pad[32n+d, k] = lora_B[n, k, d]
    b_pad = np.zeros((128, K), dtype=np.float16)
    for n in range(N):
        b_pad[ADP * n : ADP * n + D, :] = lora_B[n].T

    in_maps = []
    for c in range(N_CORES):
        # xp[chunk, half, p, pq, sub*CH + t] = x^T[(2*(8h+pq)+sub)*128 + p,
        #                                          chunk*CH + t]
        xT = xf[c * TPC : (c + 1) * TPC].T                  # [M, TPC]
        xr = xT.reshape(2, NPH, 2, 128, NCH, CH)            # [h, pq, sub, p, ch, t]
        xp = np.ascontiguousarray(xr.transpose(4, 0, 3, 1, 2, 5)).reshape(
            NCH, 2, 128, NPH, 2 * CH
        )
        in_maps.append({"xs": xp, "a_p": a_pack, "b_p": b_pad})
    return in_maps


def run(x, lora_A, lora_B, trace=False, **spmd_kwargs):
    nc = _get_nc()
    in_maps = prepare_inputs(x, lora_A, lora_B)
    res = bass_utils.run_bass_kernel_spmd(
        nc, in_maps, list(range(N_CORES)), trace=trace, **spmd_kwargs
    )
    o_full = np.concatenate(
        [res.results[c]["o"].astype(np.float32) for c in range(N_CORES)], axis=1
    )
    return o_full.reshape(N, B, J, K), res


def kernel(x, lora_A, lora_B):
    out, _ = run(x, lora_A, lora_B)
    return out


# revision 22
# speedup vs baseline: 1.1786x; 1.0341x over previous
"""Trainium2 Bass kernel for the merged multi-adapter LoRA layer.

Math (all fp32 reference):
    t[n,b,j,d]  = sum_m x[b,j,m] * lora_A[n,d,m]
    out[n,b,j,k] = sum_d t[n,b,j,d] * lora_B[n,k,d]

Shapes: x (4,2048,4096), lora_A (4,16,4096), lora_B (4,4096,16)
        out (4,4,2048,4096)

Sharding: data-parallel over flattened tokens (b*j = 8192 -> 1024/core on
8 cores); the tiny LoRA params are replicated.

Per-core HBM traffic: 8 MiB x (f16 in) + 32 MiB out (f16, widened on host)
+ ~2 MiB params  ->  ~117 us at 358 GB/s.

The PE HAM clock-gate throttles the PE to 1.2 GHz for most of the run, so
the whole kernel runs in 32x128 row-tiled PE mode (never switching mode,
so the PE never drains) and keeps the PE off the critical path:

  - mm2: the four adapters' D=16 contractions execute CONCURRENTLY on
    four 32-row PE tiles (adapter n reads t/B from SBUF partitions
    32n..32n+15 and writes its own PSUM bank) -- a group of four 512-wide
    matmuls costs ~one matmul's cycles (~131k -> ~35k PE cycles).
  - mm1: each m-tile's 128-deep contraction is split into four 32-deep
    quarter-contractions running concurrently on the four row tiles,
    accumulating four partial t tensors in four PSUM banks; a
    copy + 3 chained tensor_tensor adds on Vector reduce them to t f16.
    mm1 groups weave between mm2 groups with no mode switch.
  - PSUM: 4 x [128,512] mm2 tiles + 4 x [128,256] mm1 partials = 8 banks.
  - f32->f16 evacuation: 512-wide copies alternating Vector/Scalar.
  - x arrives pre-transposed/packed as [chunk, half, 128, 8, 512] f16,
    one 1 MiB DMA per half-chunk split across the Scalar/Sync trigger
    queues; scratch-tile warm-up matmul groups bridge the initial load.
  - stores: contiguous 0.5 MiB half-strips, first one ~6 matmul groups
    into the run.
"""

import numpy as np

import concourse.bacc as bacc
import concourse.bass as bass
import concourse.mybir as mybir
import concourse.tile as tile
from concourse import bass_utils
from concourse.bass import ds, ts

F32 = mybir.dt.float32
F16 = mybir.dt.float16

N_CORES = 8
B, J, M = 4, 2048, 4096
N, D, K = 4, 16, 4096
TOK = B * J                  # 8192 flattened tokens
TPC = TOK // N_CORES         # 1024 tokens per core
CH = 256                     # token chunk (mm1 granularity)
NCH = TPC // CH              # 4
N_MT = M // 128              # 32 m-tiles
NPAIR = N_MT // 2            # 16 packed m-tile pairs
NPH = NPAIR // 2             # pairs per half-chunk DMA (8)
KT = 512                     # mm2 matmul free width
NKG = K // KT                # 8 column groups
ADP = 32                     # partition stride per adapter / PE row tile
NSTRIP = CH // 128           # 128-token strips per chunk (2)
WARMUP = 20                  # scratch matmuls to un-throttle the PE HAM
ADD = mybir.AluOpType.add


def build_program():
    nc = bacc.Bacc("TRN2")

    xs = nc.dram_tensor(
        "xs", [NCH, 2, 128, NPH, 2 * CH], F16, kind="ExternalInput"
    ).ap()
    a_p = nc.dram_tensor("a_p", [128, N_MT, 128], F16, kind="ExternalInput").ap()
    b_p = nc.dram_tensor("b_p", [128, K], F16, kind="ExternalInput").ap()
    o = nc.dram_tensor("o", [N, TPC, K], F16, kind="ExternalOutput").ap()

    with tile.TileContext(nc) as tc:
        with (
            tc.tile_pool(name="apool", bufs=1) as apool,
            tc.tile_pool(name="bpool", bufs=1) as bpool,
            tc.tile_pool(name="spool", bufs=1) as spool,
            tc.tile_pool(name="xpool", bufs=2 * NCH) as xpool,
            tc.tile_pool(name="tpool", bufs=2) as tpool,
            tc.tile_pool(name="rpool", bufs=2) as rpool,
            tc.tile_pool(name="opool", bufs=12) as opool,
            tc.tile_pool(name="tps", bufs=4, space="PSUM") as tps_pool,
            tc.tile_pool(name="ops", bufs=4, space="PSUM") as ops_pool,
        ):
            xsb = {}
            for c in range(NCH):
                for h in range(2):
                    xsb[(c, h)] = xpool.tile([128, NPH, 2 * CH], F16, tag="x", name="x")
            a_sb = apool.tile([128, N_MT, 128], F16, tag="a")
            b_sb = bpool.tile([128, K], F16, tag="b")

            # the two halves of each chunk load in parallel on the two
            # hardware-DGE trigger queues (Scalar and Sync)
            nc.scalar.dma_start(xsb[(0, 0)][:], xs[0, 0])
            nc.sync.dma_start(a_sb[:], a_p[:])
            nc.sync.dma_start(xsb[(0, 1)][:], xs[0, 1])
            nc.scalar.dma_start(xsb[(1, 0)][:], xs[1, 0])
            nc.sync.dma_start(b_sb[:], b_p[:])
            nc.sync.dma_start(xsb[(1, 1)][:], xs[1, 1])
            nc.scalar.dma_start(xsb[(2, 0)][:], xs[2, 0])
            nc.sync.dma_start(xsb[(2, 1)][:], xs[2, 1])
            nc.scalar.dma_start(xsb[(3, 0)][:], xs[3, 0])
            nc.scalar.dma_start(xsb[(3, 1)][:], xs[3, 1])

            scr = spool.tile([128, KT], F16, tag="s", name="scr")
            nc.vector.memset(scr[:], 0.0)

            # HAM warm-up on the dependency-free scratch tile while the
            # first x chunk streams in (32x128 mode like everything else)
            for _ in range(WARMUP):
                w_ps = tps_pool.tile([128, CH], F32, tag="tps", name="wps")
                nc.tensor.matmul(
                    w_ps[:], lhsT=scr[ds(0, ADP), ds(0, 128)],
                    rhs=scr[ds(0, ADP), ds(0, CH)],
                    start=True, stop=True,
                    tile_position=(0, 0), skip_group_check=True,
                )

            def mm1_group(c, mt, parts):
                # one m-tile: 4 concurrent 32-deep quarter contractions
                for r in range(4):
                    nc.tensor.matmul(
                        parts[r][:],
                        lhsT=a_sb[ds(ADP * r, ADP), mt, :],
                        rhs=xsb[(c, mt // 16)][ds(ADP * r, ADP),
                                               (mt // 2) % NPH,
                                               ds((mt % 2) * CH, CH)],
                        start=(mt == 0),
                        stop=(mt == N_MT - 1),
                        tile_position=(ADP * r, 0),
                        skip_group_check=True,
                    )

            def mm1_parts():
                return [
                    tps_pool.tile([128, CH], F32, tag="tps", name="tps")
                    for _ in range(4)
                ]

            def mm1_reduce(parts):
                # t = p0 + p1 + p2 + p3 (one PSUM operand per op)
                red = rpool.tile([128, CH], F32, tag="r", name="red")
                nc.vector.tensor_copy(red[:], parts[0][:])
                nc.vector.tensor_add(red[:], red[:], parts[1][:])
                nc.vector.tensor_add(red[:], red[:], parts[2][:])
                t_sb = tpool.tile([128, CH], F16, tag="t", name="t")
                nc.vector.tensor_add(t_sb[:], red[:], parts[3][:])
                return t_sb

            # chunk 0's mm1 runs up front (woven against its x DMAs)
            parts = mm1_parts()
            for mt in range(N_MT):
                mm1_group(0, mt, parts)
            t_sb_next = mm1_reduce(parts)

            evac = 0
            for c in range(NCH):
                t_sb = t_sb_next
                if c + 1 < NCH:
                    parts = mm1_parts()
                    # chunk 0: weave into the back 10 mm2 groups only (its
                    # x lands mid-chunk); later chunks: spread over all 16
                    weave = {}
                    if c == 0:
                        splits = np.array_split(np.arange(N_MT), 10)
                        for g in range(6, 16):
                            weave[g] = [int(mt) for mt in splits[g - 6]]
                    else:
                        for g in range(16):
                            weave[g] = [2 * g, 2 * g + 1]

                for s in range(NSTRIP):
                    osb = [
                        opool.tile([128, K], F16, tag="o", name="osb")
                        for _ in range(N)
                    ]
                    for kg in range(NKG):
                        g = s * NKG + kg
                        o_ps = [
                            ops_pool.tile([128, KT], F32, tag="ops", name="ops")
                            for _ in range(N)
                        ]
                        # 4 adjacent matmuls on the 4 row tiles (4 distinct
                        # PSUM banks) stream concurrently
                        for n in range(N):
                            nc.tensor.matmul(
                                o_ps[n][:],
                                lhsT=t_sb[ds(ADP * n, D), ts(s, 128)],
                                rhs=b_sb[ds(ADP * n, D), ts(kg, KT)],
                                start=True,
                                stop=True,
                                tile_position=(ADP * n, 0),
                                skip_group_check=True,
                            )
                        for n in range(N):
                            if (kg + n) % 2 == 0:
                                nc.vector.tensor_copy(osb[n][:, ts(kg, KT)], o_ps[n][:])
                            else:
                                nc.scalar.copy(osb[n][:, ts(kg, KT)], o_ps[n][:])

                        # weave the next chunk's mm1 between mm2 groups
                        if c + 1 < NCH:
                            for mt in weave.get(g, []):
                                mm1_group(c + 1, mt, parts)
                                if mt == N_MT - 1:
                                    t_sb_next = mm1_reduce(parts)

                        # contiguous 0.5 MiB half-stores keep the wire busy
                        # from four matmul groups into the strip
                        if kg % 4 == 3:
                            h = kg // 4
                            for n in range(N):
                                nc.sync.dma_start(
                                    o[n, ds(c * CH + s * 128, 128),
                                      ds(h * 4 * KT, 4 * KT)],
                                    osb[n][:, ds(h * 4 * KT, 4 * KT)],
                                )

    nc.compile()
    return nc


_NC_CACHE = []


def _get_nc():
    if not _NC_CACHE:
        _NC_CACHE.append(build_program())
    return _NC_CACHE[0]


def prepare_inputs(x, lora_A, lora_B):
    x = np.ascontiguousarray(np.asarray(x, dtype=np.float32)).astype(np.float16)
    lora_A = np.asarray(lora_A, dtype=np.float32)
    lora_B = np.asarray(lora_B, dtype=np.float32)

    xf = x.reshape(TOK, M)

    # a_t[m, 32n+d] = lora_A[n, d, m]; packed to [p, mt, c] so each SBUF
    # partition reads one contiguous row.
    a_t = np.zeros((M, 128), dtype=np.float32)
    for n in range(N):
        a_t[:, ADP * n : ADP * n + D] = lora_A[n].T
    a_pack = np.ascontiguousarray(
        a_t.reshape(N_MT, 128, 128).transpose(1, 0, 2)
    ).astype(np.float16)

    # b_pad[32n+d, k] = lora_B[n, k, d]
    b_pad = np.zeros((128, K), dtype=np.float16)
    for n in range(N):
        b_pad[ADP * n : ADP * n + D, :] = lora_B[n].T

    in_maps = []
    for c in range(N_CORES):
        # xp[chunk, half, p, pq, sub*CH + t] = x^T[(2*(8h+pq)+sub)*128 + p,
        #                                          chunk*CH + t]
        xT = xf[c * TPC : (c + 1) * TPC].T                  # [M, TPC]
        xr = xT.reshape(2, NPH, 2, 128, NCH, CH)            # [h, pq, sub, p, ch, t]
        xp = np.ascontiguousarray(xr.transpose(4, 0, 3, 1, 2, 5)).reshape(
            NCH, 2, 128, NPH, 2 * CH
        )
        in_maps.append({"xs": xp, "a_p": a_pack, "b_p": b_pad})
    return in_maps


def run(x, lora_A, lora_B, trace=False, **spmd_kwargs):
    nc = _get_nc()
    in_maps = prepare_inputs(x, lora_A, lora_B)
    res = bass_utils.run_bass_kernel_spmd(
        nc, in_maps, list(range(N_CORES)), trace=trace, **spmd_kwargs
    )
    o_full = np.concatenate(
        [res.results[c]["o"].astype(np.float32) for c in range(N_CORES)], axis=1
    )
    return o_full.reshape(N, B, J, K), res


def kernel(x, lora_A, lora_B):
    out, _ = run(x, lora_A, lora_B)
    return out
